# revision 9
# baseline (speedup 1.0000x reference)
"""Trainium2 Bass kernel for periodic-masked causal self-attention.

Problem: B=4, S=2048, E=1024, H=16 heads (hd=64), fp32.
  q/k/v = hidden @ W{q,k,v}^T + b;  scores = q k^T / sqrt(hd)
  mask  = tril & ~(col % 25 == 24);  out = softmax(scores) v @ Wp^T + bp

Sharding (8 cores): data-parallel over B (4), tensor-parallel over heads
(2 groups of 8 heads = 512 feature columns). Each core computes its
batch's attention output for its 8 heads and a partial output
projection; the host sums the two partials per batch and adds bp.

Device layout (per core) — everything is kept "transposed" so no
on-chip transposes are needed anywhere:
  Q^T, K^T: [f=512, s=2048]   produced as  W^T.T @ X^T  (out [f, s])
  V:        [s=2048, f=512]   produced as  X^T.T @ Wv^T (out [s, f]),
            stored per s-block as [V_h | ones] groups of 65 cols so the
            AV matmul's lhsT rides a ones column that accumulates the
            softmax denominator.
  S^T[k,q] = (K^T_h).T @ Q^T_h  per 128-row k-block — softmax over k is
            handled by: exp on ACT (periodic mask = per-partition bias,
            1/8 scale folded in), denominator = ones column of AV lhsT,
            normalization = K=1 broadcast matmul + one multiply.
  Y[s,e]  = (O^T).T @ Wp^T  with O^T assembled pairwise [128, 2048].
"""

import sys

if "/opt/trn_rl_repo" not in sys.path:
    sys.path.insert(0, "/opt/trn_rl_repo")

import numpy as np

B, S, E, H = 4, 2048, 1024, 16
HD = 64            # head dim
FC = 512           # feature columns per core (8 heads)
NEG = -1.0e9
SCALE = 0.125      # 1/sqrt(HD)

_STATE = None      # (nc, input_names) — compile once per process


def _build_bass():
    import concourse.mybir as mybir
    import concourse.tile as tile
    import concourse.bacc as bacc

    F32 = mybir.dt.float32
    AF = mybir.ActivationFunctionType

    nc = bacc.Bacc("TRN2", target_bir_lowering=False, debug=False, num_devices=8)

    F32R = mybir.dt.float32r

    def mm(out, lhsT, rhs, start, stop):
        # operands live in float32r tiles: single-pass fp32 matmul (proper
        # fp32 runs the array twice for the HI/LO mantissa split)
        nc.tensor.matmul(out, lhsT, rhs, start=start, stop=stop)

    xT = nc.dram_tensor("xT", [E, S], F32, kind="ExternalInput")
    wqT = nc.dram_tensor("wqT", [E, FC], F32, kind="ExternalInput")
    wkT = nc.dram_tensor("wkT", [E, FC], F32, kind="ExternalInput")
    wvT = nc.dram_tensor("wvT", [E, FC], F32, kind="ExternalInput")
    wpT = nc.dram_tensor("wpT", [FC, E], F32, kind="ExternalInput")
    bqv = nc.dram_tensor("bqv", [128, 4], F32, kind="ExternalInput")
    bkv = nc.dram_tensor("bkv", [128, 4], F32, kind="ExternalInput")
    bvb = nc.dram_tensor("bvb", [128, FC], F32, kind="ExternalInput")
    trim = nc.dram_tensor("trim", [128, 128], F32, kind="ExternalInput")
    biask = nc.dram_tensor("biask", [128, 16], F32, kind="ExternalInput")
    y = nc.dram_tensor("y", [S, E], F32, kind="ExternalOutput")

    with tile.TileContext(nc) as tc:
        with (
            tc.tile_pool(name="const", bufs=1) as cpool,
            tc.tile_pool(name="qkv", bufs=1) as qkv_pool,
        ):
            trim_sb = cpool.tile([128, 128], F32, tag="trim")
            nc.gpsimd.dma_start(trim_sb[:], trim.ap())
            biask_sb = cpool.tile([128, 16], F32, tag="biask")
            nc.gpsimd.dma_start(biask_sb[:], biask.ap())
            bqv_sb = cpool.tile([128, 4], F32, tag="bqv")
            nc.gpsimd.dma_start(bqv_sb[:], bqv.ap())
            bkv_sb = cpool.tile([128, 4], F32, tag="bkv")
            nc.gpsimd.dma_start(bkv_sb[:], bkv.ap())
            bvb_sb = cpool.tile([128, FC], F32, tag="bvb")
            nc.gpsimd.dma_start(bvb_sb[:], bvb.ap())
            ones_sb = cpool.tile([128, 64], F32R, tag="ones")
            nc.scalar.activation(ones_sb[:], trim_sb[:, 0:64], AF.Identity,
                                 bias=1.0, scale=0.0)

            qT = [qkv_pool.tile([128, S], F32R, tag=f"qT{fb}", name=f"qT{fb}") for fb in range(4)]
            kT = [qkv_pool.tile([128, S], F32R, tag=f"kT{fb}", name=f"kT{fb}") for fb in range(4)]
            # V: per s-block of 128, 8 head-groups of (64 V cols + 1 ones col)
            v_sb = qkv_pool.tile([128, 16 * 520], F32R, tag="v")
            nc.scalar.activation(v_sb[:, 64 : 16 * 520 : 65], trim_sb[:, 0:128],
                                 AF.Identity, bias=1.0, scale=0.0)

            # ---------------- phase A: QKV projections ----------------
            with (
                tc.tile_pool(name="wA", bufs=1) as wpool,
                tc.tile_pool(name="xA", bufs=2) as xpool,
                tc.tile_pool(name="psA", bufs=2, space="PSUM") as psA,
            ):
                wq_sb = wpool.tile([128, 8 * FC], F32R, tag="wq")
                wk_sb = wpool.tile([128, 8 * FC], F32R, tag="wk")
                wv_sb = wpool.tile([128, 8 * FC], F32R, tag="wv")
                for ec in range(8):
                    sl = slice(512 * ec, 512 * ec + 512)
                    nc.gpsimd.dma_start(wq_sb[:, sl], wqT[128 * ec : 128 * ec + 128, :].bitcast(F32R))
                    nc.gpsimd.dma_start(wk_sb[:, sl], wkT[128 * ec : 128 * ec + 128, :].bitcast(F32R))
                    nc.gpsimd.dma_start(wv_sb[:, sl], wvT[128 * ec : 128 * ec + 128, :].bitcast(F32R))

                for sc in range(4):
                    ssl = slice(512 * sc, 512 * sc + 512)
                    xt = xpool.tile([128, 8 * 512], F32R, tag="x")
                    for ec in range(8):
                        nc.gpsimd.dma_start(
                            xt[:, 512 * ec : 512 * ec + 512],
                            xT[128 * ec : 128 * ec + 128, ssl].bitcast(F32R),
                        )
                    for fb in range(4):
                        qp = psA.tile([128, 512], F32, tag="pq")
                        kp = psA.tile([128, 512], F32, tag="pk")
                        for ec in range(8):
                            w_sl = slice(512 * ec + 128 * fb, 512 * ec + 128 * fb + 128)
                            x_sl = slice(512 * ec, 512 * ec + 512)
                            mm(
                                qp[:], wq_sb[:, w_sl], xt[:, x_sl],
                                start=(ec == 0), stop=(ec == 7),
                            )
                        for ec in range(8):
                            w_sl = slice(512 * ec + 128 * fb, 512 * ec + 128 * fb + 128)
                            x_sl = slice(512 * ec, 512 * ec + 512)
                            mm(
                                kp[:], wk_sb[:, w_sl], xt[:, x_sl],
                                start=(ec == 0), stop=(ec == 7),
                            )
                        nc.scalar.activation(
                            qT[fb][:, ssl], qp[:], AF.Identity, bias=bqv_sb[:, fb : fb + 1]
                        )
                        nc.scalar.activation(
                            kT[fb][:, ssl], kp[:], AF.Identity, bias=bkv_sb[:, fb : fb + 1]
                        )
                    for sbi in range(4):
                        sb_ = 4 * sc + sbi
                        vp = psA.tile([128, 512], F32, tag="pv")
                        for ec in range(8):
                            x_sl = slice(512 * ec + 128 * sbi, 512 * ec + 128 * sbi + 128)
                            w_sl = slice(512 * ec, 512 * ec + 512)
                            mm(
                                vp[:], xt[:, x_sl], wv_sb[:, w_sl],
                                start=(ec == 0), stop=(ec == 7),
                            )
                        vdst = v_sb[:, 520 * sb_ : 520 * sb_ + 520].rearrange(
                            "p (h c) -> p h c", c=65
                        )[:, :, 0:64]
                        nc.vector.tensor_add(
                            vdst,
                            vp[:].rearrange("p (h c) -> p h c", c=64),
                            bvb_sb[:].rearrange("p (h c) -> p h c", c=64),
                        )

            with tc.tile_pool(name="otp", bufs=1) as ot_pool:
                otp = [ot_pool.tile([128, S], F32R, tag=f"otp{fb}", name=f"otp{fb}")
                       for fb in range(4)]

                # ---------------- phase B: attention ----------------
                with (
                    tc.tile_pool(name="ptB", bufs=3) as pt_pool,
                    tc.tile_pool(name="nrmB", bufs=2) as nrm_pool,
                    tc.tile_pool(name="oddB", bufs=2) as odd_pool,
                    tc.tile_pool(name="stps", bufs=2, space="PSUM") as st_ps,
                    tc.tile_pool(name="avps", bufs=2, space="PSUM") as av_ps,
                ):
                    for h in range(8):
                        fb = h // 2
                        pb = 64 * (h % 2)
                        odd_t = None
                        if h % 2 == 1:
                            odd_t = odd_pool.tile([64, S], F32R, tag="odd")
                        for qh in range(2):
                            qbase = 1024 * qh
                            avt = [av_ps.tile([65, 512], F32, tag=f"av{cc}", name=f"av{cc}_{h}_{qh}") for cc in range(2)]
                            nkb = 8 * qh + 8
                            for kb in range(nkb):
                                k0 = 128 * kb
                                ccs = [cc for cc in (0, 1) if qbase + 512 * cc + 512 > k0]
                                st = st_ps.tile([128, 1024], F32, tag="st")
                                for cc in ccs:
                                    mm(
                                        st[:, 512 * cc : 512 * cc + 512],
                                        kT[fb][pb : pb + 64, k0 : k0 + 128],
                                        qT[fb][pb : pb + 64, qbase + 512 * cc : qbase + 512 * cc + 512],
                                        start=True, stop=True,
                                    )
                                pt = pt_pool.tile([128, 1024], F32R, tag="pt")
                                bias_ap = biask_sb[:, kb : kb + 1]
                                off = max(0, k0 - qbase)
                                if k0 >= qbase:  # diagonal band block
                                    nc.vector.tensor_add(
                                        st[:, off : off + 128], st[:, off : off + 128], trim_sb[:]
                                    )
                                nc.scalar.activation(
                                    pt[:, off:1024], st[:, off:1024], AF.Exp,
                                    bias=bias_ap, scale=SCALE,
                                )
                                for cc in ccs:
                                    # columns below the causal diagonal are never
                                    # written (psum has_written leaves them to the
                                    # other kb's matmuls)
                                    loc = min(max(off - 512 * cc, 0), 512)
                                    mm(
                                        avt[cc][:, loc:512],
                                        v_sb[:, 520 * kb + 65 * h : 520 * kb + 65 * h + 65],
                                        pt[:, 512 * cc + loc : 512 * cc + 512],
                                        start=(kb == 0),
                                        stop=(kb == 8 * qh + 4 * cc + 3),
                                    )
                            for cc in range(2):
                                scc = qbase + 512 * cc
                                rec = nrm_pool.tile([65, 512], F32R, tag="rec")
                                with nc.allow_low_precision(reason="f32r matmul operand"):
                                    nc.vector.reciprocal(rec[64:65, :], avt[cc][64:65, :])
                                bc_ps = st_ps.tile([64, 512], F32, tag="st")
                                mm(
                                    bc_ps[:], ones_sb[64:65, 0:64], rec[64:65, :],
                                    start=True, stop=True,
                                )
                                bc_sb = nrm_pool.tile([64, 512], F32, tag="bc")
                                nc.vector.tensor_copy(bc_sb[:], bc_ps[:])
                                if h % 2 == 0:
                                    nc.vector.tensor_mul(
                                        otp[fb][0:64, scc : scc + 512], avt[cc][0:64, :], bc_sb[:]
                                    )
                                else:
                                    nc.vector.tensor_mul(
                                        odd_t[0:64, scc : scc + 512], avt[cc][0:64, :], bc_sb[:]
                                    )
                        if h % 2 == 1:
                            nc.sync.dma_start(otp[fb][64:128, :], odd_t[:])

                # ---------------- phase C: output projection ----------------
                with (
                    tc.tile_pool(name="wC", bufs=1) as wpC,
                    tc.tile_pool(name="yC", bufs=3) as ypool,
                    tc.tile_pool(name="psC", bufs=2, space="PSUM") as psC,
                ):
                    wp_sb = wpC.tile([128, 4 * E], F32R, tag="wp")
                    for fc in range(4):
                        nc.gpsimd.dma_start(
                            wp_sb[:, E * fc : E * fc + E], wpT[128 * fc : 128 * fc + 128, :].bitcast(F32R)
                        )
                    for sb_ in range(16):
                        for ej in range(2):
                            yp = psC.tile([128, 512], F32, tag="y")
                            for fc in range(4):
                                mm(
                                    yp[:],
                                    otp[fc][:, 128 * sb_ : 128 * sb_ + 128],
                                    wp_sb[:, E * fc + 512 * ej : E * fc + 512 * ej + 512],
                                    start=(fc == 0), stop=(fc == 3),
                                )
                            yt = ypool.tile([128, 512], F32, tag="yt")
                            nc.vector.tensor_copy(yt[:], yp[:])
                            nc.sync.dma_start(
                                y[128 * sb_ : 128 * sb_ + 128, 512 * ej : 512 * ej + 512], yt[:]
                            )

    nc.compile()
    return nc


def _get_nc():
    global _STATE
    if _STATE is None:
        _STATE = _build_bass()
    return _STATE


def _make_in_maps(hidden, Wq, bq, Wk, bk, Wv, bv, Wp, joined_dim):
    f32 = np.float32
    ii = np.arange(128)
    trim = np.where(ii[:, None] > ii[None, :], f32(NEG), f32(0.0)).astype(f32)
    kk = (ii[:, None] + 128 * np.arange(16)[None, :])  # [128, 16] global k index
    biask = np.where(kk % joined_dim == joined_dim - 1, f32(NEG), f32(0.0)).astype(f32)

    in_maps = []
    for c in range(8):
        b = c // 2
        f0 = FC * (c % 2)
        fs = slice(f0, f0 + FC)
        in_maps.append({
            "xT": np.ascontiguousarray(hidden[b].T, dtype=f32),
            "wqT": np.ascontiguousarray(Wq[fs, :].T, dtype=f32),
            "wkT": np.ascontiguousarray(Wk[fs, :].T, dtype=f32),
            "wvT": np.ascontiguousarray(Wv[fs, :].T, dtype=f32),
            "wpT": np.ascontiguousarray(Wp[:, fs].T, dtype=f32),
            "bqv": np.ascontiguousarray(bq[fs].reshape(4, 128).T, dtype=f32),
            "bkv": np.ascontiguousarray(bk[fs].reshape(4, 128).T, dtype=f32),
            "bvb": np.ascontiguousarray(
                np.broadcast_to(bv[fs], (128, FC)), dtype=f32
            ),
            "trim": trim,
            "biask": biask,
        })
    return in_maps


def _run(in_maps, trace=False):
    from concourse.bass_utils import run_bass_kernel_spmd

    nc = _get_nc()
    return run_bass_kernel_spmd(nc, in_maps, list(range(8)), trace=trace)


def kernel(hidden_states, Wq, bq, Wk, bk, Wv, bv, Wp, bp,
           n_head, observation_dim, action_dim, **_unused):
    hidden = np.asarray(hidden_states, dtype=np.float32)
    Wq = np.asarray(Wq, dtype=np.float32)
    Wk = np.asarray(Wk, dtype=np.float32)
    Wv = np.asarray(Wv, dtype=np.float32)
    Wp = np.asarray(Wp, dtype=np.float32)
    bq = np.asarray(bq, dtype=np.float32)
    bk = np.asarray(bk, dtype=np.float32)
    bv = np.asarray(bv, dtype=np.float32)
    bp = np.asarray(bp, dtype=np.float32)
    joined_dim = int(observation_dim) + int(action_dim) + 2

    in_maps = _make_in_maps(hidden, Wq, bq, Wk, bk, Wv, bv, Wp, joined_dim)
    res = _run(in_maps)

    out = np.empty((B, S, E), dtype=np.float32)
    for b in range(B):
        out[b] = res.results[2 * b]["y"] + res.results[2 * b + 1]["y"] + bp
    return out



# revision 11
# speedup vs baseline: 1.2567x; 1.2567x over previous
"""Trainium2 Bass kernel for periodic-masked causal self-attention.

Problem: B=4, S=2048, E=1024, H=16 heads (hd=64), fp32.
  q/k/v = hidden @ W{q,k,v}^T + b;  scores = q k^T / sqrt(hd)
  mask  = tril & ~(col % 25 == 24);  out = softmax(scores) v @ Wp^T + bp

Sharding (8 cores): data-parallel over B (4), tensor-parallel over heads
(2 groups of 8 heads = 512 feature columns). Each core computes its
batch's attention output for its 8 heads and a partial output
projection; the host sums the two partials per batch and adds bp.

Device layout (per core) — everything is kept "transposed" so no
on-chip transposes are needed anywhere:
  Q^T, K^T: [f=512, s=2048]   produced as  W^T.T @ X^T  (out [f, s])
  V:        [s=2048, f=512]   produced as  X^T.T @ Wv^T (out [s, f]),
            stored per s-block as [V_h | ones] groups of 65 cols so the
            AV matmul's lhsT rides a ones column that accumulates the
            softmax denominator.
  S^T[k,q] = (K^T_h).T @ Q^T_h  per 128-row k-block — softmax over k is
            handled by: exp on ACT (periodic mask = per-partition bias,
            1/8 scale folded in), denominator = ones column of AV lhsT,
            normalization = K=1 broadcast matmul + one multiply.
  Y[s,e]  = (O^T).T @ Wp^T  with O^T assembled pairwise [128, 2048].
"""

import sys

if "/opt/trn_rl_repo" not in sys.path:
    sys.path.insert(0, "/opt/trn_rl_repo")

import numpy as np

B, S, E, H = 4, 2048, 1024, 16
HD = 64            # head dim
FC = 512           # feature columns per core (8 heads)
NEG = -1.0e9
SCALE = 0.125      # 1/sqrt(HD)

_STATE = None      # (nc, input_names) — compile once per process


def _build_bass():
    import concourse.mybir as mybir
    import concourse.tile as tile
    import concourse.bacc as bacc

    F32 = mybir.dt.float32
    AF = mybir.ActivationFunctionType

    nc = bacc.Bacc("TRN2", target_bir_lowering=False, debug=False, num_devices=8)

    F32R = mybir.dt.float32r

    def mm(out, lhsT, rhs, start, stop):
        # operands live in float32r tiles: single-pass fp32 matmul (proper
        # fp32 runs the array twice for the HI/LO mantissa split)
        nc.tensor.matmul(out, lhsT, rhs, start=start, stop=stop)

    xT = nc.dram_tensor("xT", [E, S], F32, kind="ExternalInput")
    wqT = nc.dram_tensor("wqT", [E, FC], F32, kind="ExternalInput")
    wkT = nc.dram_tensor("wkT", [E, FC], F32, kind="ExternalInput")
    wvT = nc.dram_tensor("wvT", [E, FC], F32, kind="ExternalInput")
    wpT = nc.dram_tensor("wpT", [FC, E], F32, kind="ExternalInput")
    bqv = nc.dram_tensor("bqv", [128, 4], F32, kind="ExternalInput")
    bkv = nc.dram_tensor("bkv", [128, 4], F32, kind="ExternalInput")
    bvb = nc.dram_tensor("bvb", [128, FC], F32, kind="ExternalInput")
    trim = nc.dram_tensor("trim", [128, 128], F32, kind="ExternalInput")
    biask = nc.dram_tensor("biask", [128, 16], F32, kind="ExternalInput")
    lrec = nc.dram_tensor("lrec", [8, 2, 2, 512], F32)
    y = nc.dram_tensor("y", [S, E], F32, kind="ExternalOutput")

    with tile.TileContext(nc) as tc:
        with (
            tc.tile_pool(name="const", bufs=1) as cpool,
            tc.tile_pool(name="qkv", bufs=1) as qkv_pool,
        ):
            trim_sb = cpool.tile([128, 128], F32, tag="trim")
            nc.gpsimd.dma_start(trim_sb[:], trim.ap())
            biask_sb = cpool.tile([128, 16], F32, tag="biask")
            nc.gpsimd.dma_start(biask_sb[:], biask.ap())
            bqv_sb = cpool.tile([128, 4], F32, tag="bqv")
            nc.gpsimd.dma_start(bqv_sb[:], bqv.ap())
            bkv_sb = cpool.tile([128, 4], F32, tag="bkv")
            nc.gpsimd.dma_start(bkv_sb[:], bkv.ap())
            bvb_sb = cpool.tile([128, FC], F32, tag="bvb")
            nc.gpsimd.dma_start(bvb_sb[:], bvb.ap())

            qT = [qkv_pool.tile([128, S], F32R, tag=f"qT{fb}", name=f"qT{fb}") for fb in range(4)]
            kT = [qkv_pool.tile([128, S], F32R, tag=f"kT{fb}", name=f"kT{fb}") for fb in range(4)]
            # V: per s-block of 128, 8 head-groups of (64 V cols + 1 ones col)
            v_sb = qkv_pool.tile([128, 16 * 520], F32R, tag="v")
            nc.scalar.activation(v_sb[:, 64 : 16 * 520 : 65], trim_sb[:, 0:128],
                                 AF.Identity, bias=1.0, scale=0.0)

            # ---------------- phase A: QKV projections ----------------
            with (
                tc.tile_pool(name="wA", bufs=1) as wpool,
                tc.tile_pool(name="xA", bufs=2) as xpool,
                tc.tile_pool(name="psA", bufs=2, space="PSUM") as psA,
            ):
                wq_sb = wpool.tile([128, 8 * FC], F32R, tag="wq")
                wk_sb = wpool.tile([128, 8 * FC], F32R, tag="wk")
                wv_sb = wpool.tile([128, 8 * FC], F32R, tag="wv")
                for ec in range(8):
                    sl = slice(512 * ec, 512 * ec + 512)
                    nc.gpsimd.dma_start(wq_sb[:, sl], wqT[128 * ec : 128 * ec + 128, :].bitcast(F32R))
                    nc.gpsimd.dma_start(wk_sb[:, sl], wkT[128 * ec : 128 * ec + 128, :].bitcast(F32R))
                    nc.gpsimd.dma_start(wv_sb[:, sl], wvT[128 * ec : 128 * ec + 128, :].bitcast(F32R))

                for sc in range(4):
                    ssl = slice(512 * sc, 512 * sc + 512)
                    xt = xpool.tile([128, 8 * 512], F32R, tag="x")
                    for ec in range(8):
                        nc.gpsimd.dma_start(
                            xt[:, 512 * ec : 512 * ec + 512],
                            xT[128 * ec : 128 * ec + 128, ssl].bitcast(F32R),
                        )
                    for fb in range(4):
                        qp = psA.tile([128, 512], F32, tag="pq")
                        kp = psA.tile([128, 512], F32, tag="pk")
                        for ec in range(8):
                            w_sl = slice(512 * ec + 128 * fb, 512 * ec + 128 * fb + 128)
                            x_sl = slice(512 * ec, 512 * ec + 512)
                            mm(
                                qp[:], wq_sb[:, w_sl], xt[:, x_sl],
                                start=(ec == 0), stop=(ec == 7),
                            )
                        for ec in range(8):
                            w_sl = slice(512 * ec + 128 * fb, 512 * ec + 128 * fb + 128)
                            x_sl = slice(512 * ec, 512 * ec + 512)
                            mm(
                                kp[:], wk_sb[:, w_sl], xt[:, x_sl],
                                start=(ec == 0), stop=(ec == 7),
                            )
                        nc.scalar.activation(
                            qT[fb][:, ssl], qp[:], AF.Identity, bias=bqv_sb[:, fb : fb + 1]
                        )
                        nc.scalar.activation(
                            kT[fb][:, ssl], kp[:], AF.Identity, bias=bkv_sb[:, fb : fb + 1]
                        )
                    for sbi in range(4):
                        sb_ = 4 * sc + sbi
                        vp = psA.tile([128, 512], F32, tag="pv")
                        for ec in range(8):
                            x_sl = slice(512 * ec + 128 * sbi, 512 * ec + 128 * sbi + 128)
                            w_sl = slice(512 * ec, 512 * ec + 512)
                            mm(
                                vp[:], xt[:, x_sl], wv_sb[:, w_sl],
                                start=(ec == 0), stop=(ec == 7),
                            )
                        vdst = v_sb[:, 520 * sb_ : 520 * sb_ + 520].rearrange(
                            "p (h c) -> p h c", c=65
                        )[:, :, 0:64]
                        nc.vector.tensor_add(
                            vdst,
                            vp[:].rearrange("p (h c) -> p h c", c=64),
                            bvb_sb[:].rearrange("p (h c) -> p h c", c=64),
                        )

            with tc.tile_pool(name="otp", bufs=1) as ot_pool:
                otp = [ot_pool.tile([128, S], F32R, tag=f"otp{fb}", name=f"otp{fb}")
                       for fb in range(4)]

                # ---------------- phase B: attention ----------------
                with (
                    tc.tile_pool(name="ptB", bufs=3) as pt_pool,
                    tc.tile_pool(name="nrmB", bufs=2) as nrm_pool,
                    tc.tile_pool(name="oddB", bufs=2) as odd_pool,
                    tc.tile_pool(name="stps", bufs=2, space="PSUM") as st_ps,
                    tc.tile_pool(name="avps", bufs=2, space="PSUM") as av_ps,
                ):
                    for h in range(8):
                        fb = h // 2
                        pb = 64 * (h % 2)
                        odd_t = None
                        if h % 2 == 1:
                            odd_t = odd_pool.tile([64, S], F32R, tag="odd")
                        for qh in range(2):
                            qbase = 1024 * qh
                            avt = [av_ps.tile([65, 512], F32, tag=f"av{cc}", name=f"av{cc}_{h}_{qh}") for cc in range(2)]
                            nkb = 8 * qh + 8
                            for kb in range(nkb):
                                k0 = 128 * kb
                                ccs = [cc for cc in (0, 1) if qbase + 512 * cc + 512 > k0]
                                st = st_ps.tile([128, 1024], F32, tag="st")
                                for cc in ccs:
                                    mm(
                                        st[:, 512 * cc : 512 * cc + 512],
                                        kT[fb][pb : pb + 64, k0 : k0 + 128],
                                        qT[fb][pb : pb + 64, qbase + 512 * cc : qbase + 512 * cc + 512],
                                        start=True, stop=True,
                                    )
                                pt = pt_pool.tile([128, 1024], F32R, tag="pt")
                                bias_ap = biask_sb[:, kb : kb + 1]
                                off = max(0, k0 - qbase)
                                if k0 >= qbase:  # diagonal band block
                                    nc.vector.tensor_add(
                                        st[:, off : off + 128], st[:, off : off + 128], trim_sb[:]
                                    )
                                nc.scalar.activation(
                                    pt[:, off:1024], st[:, off:1024], AF.Exp,
                                    bias=bias_ap, scale=SCALE,
                                )
                                for cc in ccs:
                                    # columns below the causal diagonal are never
                                    # written (psum has_written leaves them to the
                                    # other kb's matmuls)
                                    loc = min(max(off - 512 * cc, 0), 512)
                                    mm(
                                        avt[cc][:, loc:512],
                                        v_sb[:, 520 * kb + 65 * h : 520 * kb + 65 * h + 65],
                                        pt[:, 512 * cc + loc : 512 * cc + 512],
                                        start=(kb == 0),
                                        stop=(kb == 8 * qh + 4 * cc + 3),
                                    )
                            for cc in range(2):
                                scc = qbase + 512 * cc
                                lnl = nrm_pool.tile([65, 512], F32, tag="lnl")
                                nc.scalar.activation(lnl[64:65, :], avt[cc][64:65, :], AF.Ln)
                                rec = nrm_pool.tile([65, 512], F32, tag="rec")
                                nc.scalar.activation(rec[64:65, :], lnl[64:65, :], AF.Exp,
                                                     scale=-1.0)
                                nc.sync.dma_start(lrec[h, qh, cc, :], rec[64:65, :])
                                bc_sb = nrm_pool.tile([64, 512], F32, tag="bc")
                                nc.sync.dma_start(bc_sb[:], lrec[h, qh, cc : cc + 1, :].to_broadcast((64, 512)))
                                if h % 2 == 0:
                                    nc.vector.tensor_mul(
                                        otp[fb][0:64, scc : scc + 512], avt[cc][0:64, :], bc_sb[:]
                                    )
                                else:
                                    nc.vector.tensor_mul(
                                        odd_t[0:64, scc : scc + 512], avt[cc][0:64, :], bc_sb[:]
                                    )
                        if h % 2 == 1:
                            nc.sync.dma_start(otp[fb][64:128, :], odd_t[:])

                # ---------------- phase C: output projection ----------------
                with (
                    tc.tile_pool(name="wC", bufs=1) as wpC,
                    tc.tile_pool(name="yC", bufs=3) as ypool,
                    tc.tile_pool(name="psC", bufs=2, space="PSUM") as psC,
                ):
                    wp_sb = wpC.tile([128, 4 * E], F32R, tag="wp")
                    for fc in range(4):
                        nc.gpsimd.dma_start(
                            wp_sb[:, E * fc : E * fc + E], wpT[128 * fc : 128 * fc + 128, :].bitcast(F32R)
                        )
                    for sb_ in range(16):
                        for ej in range(2):
                            yp = psC.tile([128, 512], F32, tag="y")
                            for fc in range(4):
                                mm(
                                    yp[:],
                                    otp[fc][:, 128 * sb_ : 128 * sb_ + 128],
                                    wp_sb[:, E * fc + 512 * ej : E * fc + 512 * ej + 512],
                                    start=(fc == 0), stop=(fc == 3),
                                )
                            yt = ypool.tile([128, 512], F32, tag="yt")
                            nc.vector.tensor_copy(yt[:], yp[:])
                            nc.sync.dma_start(
                                y[128 * sb_ : 128 * sb_ + 128, 512 * ej : 512 * ej + 512], yt[:]
                            )

    nc.compile()
    return nc


def _get_nc():
    global _STATE
    if _STATE is None:
        _STATE = _build_bass()
    return _STATE


def _make_in_maps(hidden, Wq, bq, Wk, bk, Wv, bv, Wp, joined_dim):
    f32 = np.float32
    ii = np.arange(128)
    trim = np.where(ii[:, None] > ii[None, :], f32(NEG), f32(0.0)).astype(f32)
    kk = (ii[:, None] + 128 * np.arange(16)[None, :])  # [128, 16] global k index
    biask = np.where(kk % joined_dim == joined_dim - 1, f32(NEG), f32(0.0)).astype(f32)

    in_maps = []
    for c in range(8):
        b = c // 2
        f0 = FC * (c % 2)
        fs = slice(f0, f0 + FC)
        in_maps.append({
            "xT": np.ascontiguousarray(hidden[b].T, dtype=f32),
            "wqT": np.ascontiguousarray(Wq[fs, :].T, dtype=f32),
            "wkT": np.ascontiguousarray(Wk[fs, :].T, dtype=f32),
            "wvT": np.ascontiguousarray(Wv[fs, :].T, dtype=f32),
            "wpT": np.ascontiguousarray(Wp[:, fs].T, dtype=f32),
            "bqv": np.ascontiguousarray(bq[fs].reshape(4, 128).T, dtype=f32),
            "bkv": np.ascontiguousarray(bk[fs].reshape(4, 128).T, dtype=f32),
            "bvb": np.ascontiguousarray(
                np.broadcast_to(bv[fs], (128, FC)), dtype=f32
            ),
            "trim": trim,
            "biask": biask,
        })
    return in_maps


def _run(in_maps, trace=False):
    from concourse.bass_utils import run_bass_kernel_spmd

    nc = _get_nc()
    return run_bass_kernel_spmd(nc, in_maps, list(range(8)), trace=trace)


def kernel(hidden_states, Wq, bq, Wk, bk, Wv, bv, Wp, bp,
           n_head, observation_dim, action_dim, **_unused):
    hidden = np.asarray(hidden_states, dtype=np.float32)
    Wq = np.asarray(Wq, dtype=np.float32)
    Wk = np.asarray(Wk, dtype=np.float32)
    Wv = np.asarray(Wv, dtype=np.float32)
    Wp = np.asarray(Wp, dtype=np.float32)
    bq = np.asarray(bq, dtype=np.float32)
    bk = np.asarray(bk, dtype=np.float32)
    bv = np.asarray(bv, dtype=np.float32)
    bp = np.asarray(bp, dtype=np.float32)
    joined_dim = int(observation_dim) + int(action_dim) + 2

    in_maps = _make_in_maps(hidden, Wq, bq, Wk, bk, Wv, bv, Wp, joined_dim)
    res = _run(in_maps)

    out = np.empty((B, S, E), dtype=np.float32)
    for b in range(B):
        out[b] = res.results[2 * b]["y"] + res.results[2 * b + 1]["y"] + bp
    return out



# revision 12
# speedup vs baseline: 1.3244x; 1.0539x over previous
"""Trainium2 Bass kernel for periodic-masked causal self-attention.

Problem: B=4, S=2048, E=1024, H=16 heads (hd=64), fp32.
  q/k/v = hidden @ W{q,k,v}^T + b;  scores = q k^T / sqrt(hd)
  mask  = tril & ~(col % 25 == 24);  out = softmax(scores) v @ Wp^T + bp

Sharding (8 cores): data-parallel over B (4), tensor-parallel over heads
(2 groups of 8 heads = 512 feature columns). Each core computes its
batch's attention output for its 8 heads and a partial output
projection; the host sums the two partials per batch and adds bp.

Device layout (per core) — everything is kept "transposed" so no
on-chip transposes are needed anywhere:
  Q^T, K^T: [f=512, s=2048]   produced as  W^T.T @ X^T  (out [f, s])
  V:        [s=2048, f=512]   produced as  X^T.T @ Wv^T (out [s, f]),
            stored per s-block as [V_h | ones] groups of 65 cols so the
            AV matmul's lhsT rides a ones column that accumulates the
            softmax denominator.
  S^T[k,q] = (K^T_h).T @ Q^T_h  per 128-row k-block — softmax over k is
            handled by: exp on ACT (periodic mask = per-partition bias,
            1/8 scale folded in), denominator = ones column of AV lhsT,
            normalization = K=1 broadcast matmul + one multiply.
  Y[s,e]  = (O^T).T @ Wp^T  with O^T assembled pairwise [128, 2048].
"""

import sys

if "/opt/trn_rl_repo" not in sys.path:
    sys.path.insert(0, "/opt/trn_rl_repo")

import numpy as np

B, S, E, H = 4, 2048, 1024, 16
HD = 64            # head dim
FC = 512           # feature columns per core (8 heads)
NEG = -1.0e9
SCALE = 0.125      # 1/sqrt(HD)

_STATE = None      # (nc, input_names) — compile once per process


def _build_bass():
    import concourse.mybir as mybir
    import concourse.tile as tile
    import concourse.bacc as bacc

    F32 = mybir.dt.float32
    AF = mybir.ActivationFunctionType

    nc = bacc.Bacc("TRN2", target_bir_lowering=False, debug=False, num_devices=8)

    F32R = mybir.dt.float32r

    def mm(out, lhsT, rhs, start, stop):
        # operands live in float32r tiles: single-pass fp32 matmul (proper
        # fp32 runs the array twice for the HI/LO mantissa split)
        nc.tensor.matmul(out, lhsT, rhs, start=start, stop=stop)

    xT = nc.dram_tensor("xT", [E, S], F32, kind="ExternalInput")
    wqT = nc.dram_tensor("wqT", [E, FC], F32, kind="ExternalInput")
    wkT = nc.dram_tensor("wkT", [E, FC], F32, kind="ExternalInput")
    wvT = nc.dram_tensor("wvT", [E, FC], F32, kind="ExternalInput")
    wpT = nc.dram_tensor("wpT", [FC, E], F32, kind="ExternalInput")
    bqv = nc.dram_tensor("bqv", [128, 4], F32, kind="ExternalInput")
    bkv = nc.dram_tensor("bkv", [128, 4], F32, kind="ExternalInput")
    bvb = nc.dram_tensor("bvb", [128, FC], F32, kind="ExternalInput")
    trim = nc.dram_tensor("trim", [128, 128], F32, kind="ExternalInput")
    biask = nc.dram_tensor("biask", [128, 16], F32, kind="ExternalInput")
    lrec = nc.dram_tensor("lrec", [8, 2, 2, 512], F32)
    y = nc.dram_tensor("y", [S, E], F32, kind="ExternalOutput")

    with tile.TileContext(nc) as tc:
        with (
            tc.tile_pool(name="const", bufs=1) as cpool,
            tc.tile_pool(name="qkv", bufs=1) as qkv_pool,
        ):
            trim_sb = cpool.tile([128, 128], F32, tag="trim")
            nc.gpsimd.dma_start(trim_sb[:], trim.ap())
            biask_sb = cpool.tile([128, 16], F32, tag="biask")
            nc.gpsimd.dma_start(biask_sb[:], biask.ap())
            bqv_sb = cpool.tile([128, 4], F32, tag="bqv")
            nc.gpsimd.dma_start(bqv_sb[:], bqv.ap())
            bkv_sb = cpool.tile([128, 4], F32, tag="bkv")
            nc.gpsimd.dma_start(bkv_sb[:], bkv.ap())
            bvb_sb = cpool.tile([128, FC], F32, tag="bvb")
            nc.gpsimd.dma_start(bvb_sb[:], bvb.ap())

            qT = [qkv_pool.tile([128, S], F32R, tag=f"qT{fb}", name=f"qT{fb}") for fb in range(4)]
            kT = [qkv_pool.tile([128, S], F32R, tag=f"kT{fb}", name=f"kT{fb}") for fb in range(4)]
            # V: per s-block of 128, 8 head-groups of (64 V cols + 1 ones col)
            v_sb = qkv_pool.tile([128, 16 * 520], F32R, tag="v")
            nc.scalar.activation(v_sb[:, 64 : 16 * 520 : 65], trim_sb[:, 0:128],
                                 AF.Identity, bias=1.0, scale=0.0)

            # ---------------- phase A: QKV projections ----------------
            with (
                tc.tile_pool(name="wA", bufs=1) as wpool,
                tc.tile_pool(name="xA", bufs=2) as xpool,
                tc.tile_pool(name="psA", bufs=2, space="PSUM") as psA,
            ):
                wq_sb = wpool.tile([128, 8 * FC], F32R, tag="wq")
                wk_sb = wpool.tile([128, 8 * FC], F32R, tag="wk")
                wv_sb = wpool.tile([128, 8 * FC], F32R, tag="wv")
                for ec in range(8):
                    sl = slice(512 * ec, 512 * ec + 512)
                    nc.gpsimd.dma_start(wq_sb[:, sl], wqT[128 * ec : 128 * ec + 128, :].bitcast(F32R))
                    nc.gpsimd.dma_start(wk_sb[:, sl], wkT[128 * ec : 128 * ec + 128, :].bitcast(F32R))
                    nc.gpsimd.dma_start(wv_sb[:, sl], wvT[128 * ec : 128 * ec + 128, :].bitcast(F32R))

                for sc in range(4):
                    ssl = slice(512 * sc, 512 * sc + 512)
                    xt = xpool.tile([128, 8 * 512], F32R, tag="x")
                    for ec in range(8):
                        nc.gpsimd.dma_start(
                            xt[:, 512 * ec : 512 * ec + 512],
                            xT[128 * ec : 128 * ec + 128, ssl].bitcast(F32R),
                        )
                    for fb in range(4):
                        qp = psA.tile([128, 512], F32, tag="pq")
                        kp = psA.tile([128, 512], F32, tag="pk")
                        for ec in range(8):
                            w_sl = slice(512 * ec + 128 * fb, 512 * ec + 128 * fb + 128)
                            x_sl = slice(512 * ec, 512 * ec + 512)
                            mm(
                                qp[:], wq_sb[:, w_sl], xt[:, x_sl],
                                start=(ec == 0), stop=(ec == 7),
                            )
                        for ec in range(8):
                            w_sl = slice(512 * ec + 128 * fb, 512 * ec + 128 * fb + 128)
                            x_sl = slice(512 * ec, 512 * ec + 512)
                            mm(
                                kp[:], wk_sb[:, w_sl], xt[:, x_sl],
                                start=(ec == 0), stop=(ec == 7),
                            )
                        nc.scalar.activation(
                            qT[fb][:, ssl], qp[:], AF.Identity, bias=bqv_sb[:, fb : fb + 1]
                        )
                        nc.scalar.activation(
                            kT[fb][:, ssl], kp[:], AF.Identity, bias=bkv_sb[:, fb : fb + 1]
                        )
                    for sbi in range(4):
                        sb_ = 4 * sc + sbi
                        vp = psA.tile([128, 512], F32, tag="pv")
                        for ec in range(8):
                            x_sl = slice(512 * ec + 128 * sbi, 512 * ec + 128 * sbi + 128)
                            w_sl = slice(512 * ec, 512 * ec + 512)
                            mm(
                                vp[:], xt[:, x_sl], wv_sb[:, w_sl],
                                start=(ec == 0), stop=(ec == 7),
                            )
                        vdst = v_sb[:, 520 * sb_ : 520 * sb_ + 520].rearrange(
                            "p (h c) -> p h c", c=65
                        )[:, :, 0:64]
                        nc.vector.tensor_add(
                            vdst,
                            vp[:].rearrange("p (h c) -> p h c", c=64),
                            bvb_sb[:].rearrange("p (h c) -> p h c", c=64),
                        )

            with tc.tile_pool(name="otp", bufs=1) as ot_pool:
                otp = [ot_pool.tile([128, S], F32R, tag=f"otp{fb}", name=f"otp{fb}")
                       for fb in range(4)]

                # ---------------- phase B: attention ----------------
                with (
                    tc.tile_pool(name="ptB", bufs=3) as pt_pool,
                    tc.tile_pool(name="nrmB", bufs=2) as nrm_pool,
                    tc.tile_pool(name="oddB", bufs=2) as odd_pool,
                    tc.tile_pool(name="stps", bufs=2, space="PSUM") as st_ps,
                    tc.tile_pool(name="avps", bufs=2, space="PSUM") as av_ps,
                ):
                    for h in range(8):
                        fb = h // 2
                        pb = 64 * (h % 2)
                        odd_t = None
                        if h % 2 == 1:
                            odd_t = odd_pool.tile([64, S], F32R, tag="odd")
                        for qh in range(2):
                            qbase = 1024 * qh
                            avt = [av_ps.tile([65, 512], F32, tag=f"av{cc}", name=f"av{cc}_{h}_{qh}") for cc in range(2)]
                            nkb = 8 * qh + 8
                            for kb in range(nkb):
                                k0 = 128 * kb
                                ccs = [cc for cc in (0, 1) if qbase + 512 * cc + 512 > k0]
                                st = st_ps.tile([128, 1024], F32, tag="st")
                                for cc in ccs:
                                    mm(
                                        st[:, 512 * cc : 512 * cc + 512],
                                        kT[fb][pb : pb + 64, k0 : k0 + 128],
                                        qT[fb][pb : pb + 64, qbase + 512 * cc : qbase + 512 * cc + 512],
                                        start=True, stop=True,
                                    )
                                pt = pt_pool.tile([128, 1024], F32R, tag="pt")
                                bias_ap = biask_sb[:, kb : kb + 1]
                                off = max(0, k0 - qbase)
                                if k0 >= qbase:  # diagonal band block
                                    nc.vector.tensor_add(
                                        st[:, off : off + 128], st[:, off : off + 128], trim_sb[:]
                                    )
                                nc.scalar.activation(
                                    pt[:, off:1024], st[:, off:1024], AF.Exp,
                                    bias=bias_ap, scale=SCALE,
                                )
                                for cc in ccs:
                                    # columns below the causal diagonal are never
                                    # written (psum has_written leaves them to the
                                    # other kb's matmuls)
                                    loc = min(max(off - 512 * cc, 0), 512)
                                    mm(
                                        avt[cc][:, loc:512],
                                        v_sb[:, 520 * kb + 65 * h : 520 * kb + 65 * h + 65],
                                        pt[:, 512 * cc + loc : 512 * cc + 512],
                                        start=(kb == 0),
                                        stop=(kb == 8 * qh + 4 * cc + 3),
                                    )
                            for cc in range(2):
                                scc = qbase + 512 * cc
                                rec = nrm_pool.tile([65, 512], F32, tag="rec")
                                nc.vector.reciprocal(rec[64:65, :], avt[cc][64:65, :])
                                nc.sync.dma_start(lrec[h, qh, cc, :], rec[64:65, :])
                                bc_sb = nrm_pool.tile([64, 512], F32, tag="bc")
                                nc.sync.dma_start(bc_sb[:], lrec[h, qh, cc : cc + 1, :].to_broadcast((64, 512)))
                                if h % 2 == 0:
                                    nc.vector.tensor_mul(
                                        otp[fb][0:64, scc : scc + 512], avt[cc][0:64, :], bc_sb[:]
                                    )
                                else:
                                    nc.vector.tensor_mul(
                                        odd_t[0:64, scc : scc + 512], avt[cc][0:64, :], bc_sb[:]
                                    )
                        if h % 2 == 1:
                            nc.sync.dma_start(otp[fb][64:128, :], odd_t[:])

                # ---------------- phase C: output projection ----------------
                with (
                    tc.tile_pool(name="wC", bufs=1) as wpC,
                    tc.tile_pool(name="yC", bufs=3) as ypool,
                    tc.tile_pool(name="psC", bufs=2, space="PSUM") as psC,
                ):
                    wp_sb = wpC.tile([128, 4 * E], F32R, tag="wp")
                    for fc in range(4):
                        nc.gpsimd.dma_start(
                            wp_sb[:, E * fc : E * fc + E], wpT[128 * fc : 128 * fc + 128, :].bitcast(F32R)
                        )
                    for sb_ in range(16):
                        for ej in range(2):
                            yp = psC.tile([128, 512], F32, tag="y")
                            for fc in range(4):
                                mm(
                                    yp[:],
                                    otp[fc][:, 128 * sb_ : 128 * sb_ + 128],
                                    wp_sb[:, E * fc + 512 * ej : E * fc + 512 * ej + 512],
                                    start=(fc == 0), stop=(fc == 3),
                                )
                            yt = ypool.tile([128, 512], F32, tag="yt")
                            nc.vector.tensor_copy(yt[:], yp[:])
                            nc.sync.dma_start(
                                y[128 * sb_ : 128 * sb_ + 128, 512 * ej : 512 * ej + 512], yt[:]
                            )

    nc.compile()
    return nc


def _get_nc():
    global _STATE
    if _STATE is None:
        _STATE = _build_bass()
    return _STATE


def _make_in_maps(hidden, Wq, bq, Wk, bk, Wv, bv, Wp, joined_dim):
    f32 = np.float32
    ii = np.arange(128)
    trim = np.where(ii[:, None] > ii[None, :], f32(NEG), f32(0.0)).astype(f32)
    kk = (ii[:, None] + 128 * np.arange(16)[None, :])  # [128, 16] global k index
    biask = np.where(kk % joined_dim == joined_dim - 1, f32(NEG), f32(0.0)).astype(f32)

    in_maps = []
    for c in range(8):
        b = c // 2
        f0 = FC * (c % 2)
        fs = slice(f0, f0 + FC)
        in_maps.append({
            "xT": np.ascontiguousarray(hidden[b].T, dtype=f32),
            "wqT": np.ascontiguousarray(Wq[fs, :].T, dtype=f32),
            "wkT": np.ascontiguousarray(Wk[fs, :].T, dtype=f32),
            "wvT": np.ascontiguousarray(Wv[fs, :].T, dtype=f32),
            "wpT": np.ascontiguousarray(Wp[:, fs].T, dtype=f32),
            "bqv": np.ascontiguousarray(bq[fs].reshape(4, 128).T, dtype=f32),
            "bkv": np.ascontiguousarray(bk[fs].reshape(4, 128).T, dtype=f32),
            "bvb": np.ascontiguousarray(
                np.broadcast_to(bv[fs], (128, FC)), dtype=f32
            ),
            "trim": trim,
            "biask": biask,
        })
    return in_maps


def _run(in_maps, trace=False):
    from concourse.bass_utils import run_bass_kernel_spmd

    nc = _get_nc()
    return run_bass_kernel_spmd(nc, in_maps, list(range(8)), trace=trace)


def kernel(hidden_states, Wq, bq, Wk, bk, Wv, bv, Wp, bp,
           n_head, observation_dim, action_dim, **_unused):
    hidden = np.asarray(hidden_states, dtype=np.float32)
    Wq = np.asarray(Wq, dtype=np.float32)
    Wk = np.asarray(Wk, dtype=np.float32)
    Wv = np.asarray(Wv, dtype=np.float32)
    Wp = np.asarray(Wp, dtype=np.float32)
    bq = np.asarray(bq, dtype=np.float32)
    bk = np.asarray(bk, dtype=np.float32)
    bv = np.asarray(bv, dtype=np.float32)
    bp = np.asarray(bp, dtype=np.float32)
    joined_dim = int(observation_dim) + int(action_dim) + 2

    in_maps = _make_in_maps(hidden, Wq, bq, Wk, bk, Wv, bv, Wp, joined_dim)
    res = _run(in_maps)

    out = np.empty((B, S, E), dtype=np.float32)
    for b in range(B):
        out[b] = res.results[2 * b]["y"] + res.results[2 * b + 1]["y"] + bp
    return out

